# revision 18
# baseline (speedup 1.0000x reference)
"""Sparse cross-attention (squared-ReLU normalizer) on 8 TRN2 NeuronCores.

Sharding: 8 cores = batch(2) x head-group(4). Each core owns one batch and
4 of 16 heads (a 256-wide slice of hsize): Wq/Wkv column-parallel,
Wo row-parallel (partial outputs summed on host), mask replicated per
batch shard.

Per-core kernel, engine-balanced (bf16 matmuls, fp32 PSUM):
  stage A: rqT (hs,q), rkT (hs,s) via weight-stationary projections into
    [128,1024] PSUM tiles; rv (s, hs+ones) with kT chunks as weights.
    rq evicted on ACT (scale 1/sqrt(a) folded), rk/rv evicted on DVE.
  middle, per (q-tile 512, head): 8 score-matmul pairs -> [128,1024] PSUM,
    relu+nbias on ACT -> u (bf16); u^2 on DVE tensor_scalar pow (4x mode);
    t = u^2 * maskT on DVE/GpSimd; AV with rv chunks as weights gives
    oT (64,q) + denominator row; reciprocal on DVE; PE ones-outer-product
    broadcasts rec; DVE normalizes+evicts oT.
  tail: out[q,1024] partial = oT^T @ woT accumulated in PSUM, DMA'd
    directly PSUM->DRAM.
"""

import numpy as np
import ml_dtypes

BF16 = ml_dtypes.bfloat16

B, Q, S, D = 2, 2048, 2048, 1024
NUM_HEAD, ADIM = 16, 64
HSIZE = NUM_HEAD * ADIM
N_CORES = 8
GROUPS = 4                  # head groups (tensor-parallel dim)
HPG = NUM_HEAD // GROUPS    # 4 heads per core
HS = HPG * ADIM             # 256: per-core hsize slice
P = 128

USE_POW = False             # DVE pow fails the ISA check; square via tensor_mul
POOL_CHUNKS = (3,)          # which 4-sc chunks' mask-mul goes to GpSimd
                            # (last chunk: its lag hides behind the next head)

_COMPILED = None


def _build(q=Q, s=S, d=D, hpg=HPG, adim=ADIM, qt=512):
    """Build + compile the per-core Bass program. Returns the Bacc."""
    from contextlib import ExitStack
    import concourse.bass as bass
    import concourse.mybir as mybir
    import concourse.tile as tile
    from concourse import bacc

    fp32 = mybir.dt.float32
    bf16 = mybir.dt.bfloat16
    AF = mybir.ActivationFunctionType
    ALU = mybir.AluOpType

    hs = hpg * adim
    DC = d // P          # contraction chunks for projections (8)
    NQ = q // qt         # q tiles (4)
    SC = s // P          # s chunks (16)
    HC = hs // P         # hsize-slice chunks (2)
    assert hs % P == 0 and q % qt == 0 and qt == 512 and d == 1024

    nc = bacc.Bacc("TRN2", target_bir_lowering=False, debug=False,
                   num_devices=N_CORES)

    qT = nc.dram_tensor("qT", [d, q], bf16, kind="ExternalInput").ap()
    kT = nc.dram_tensor("kT", [d, s], bf16, kind="ExternalInput").ap()
    wqT = nc.dram_tensor("wqT", [d, hs], bf16, kind="ExternalInput").ap()
    wkT = nc.dram_tensor("wkT", [d, hs], bf16, kind="ExternalInput").ap()
    wvT = nc.dram_tensor("wvT", [d, hs], bf16, kind="ExternalInput").ap()
    woT = nc.dram_tensor("woT", [hs, d], bf16, kind="ExternalInput").ap()
    maskT = nc.dram_tensor("maskT", [s, q], bf16, kind="ExternalInput").ap()
    nbias = nc.dram_tensor("nbias", [1, 1], fp32, kind="ExternalInput").ap()
    out = nc.dram_tensor("out", [q, d], bf16, kind="ExternalOutput").ap()

    qT_t = qT.rearrange("(c p) q -> c p q", p=P)        # [8, 128, q]
    kT_t = kT.rearrange("(c p) s -> c p s", p=P)
    wqT_t = wqT.rearrange("(c p) h -> c p h", p=P)
    wkT_t = wkT.rearrange("(c p) h -> c p h", p=P)
    wvT_t = wvT.rearrange("(c p) h -> c p h", p=P)
    woT_t = woT.rearrange("(c p) d -> c p d", p=P)      # [2, 128, d]
    maskT_t = maskT.rearrange("(c p) q -> p c q", p=P)  # [128, SC, q]
    out_t = out.rearrange("(c p) d -> c p d", p=P)      # [q/P, 128, d]

    scale = 1.0 / np.sqrt(np.float32(adim))

    with tile.TileContext(nc) as tc, ExitStack() as ctx:
        const = ctx.enter_context(tc.tile_pool(name="const", bufs=1))
        wpool = ctx.enter_context(tc.tile_pool(name="w", bufs=1))
        xpool = ctx.enter_context(tc.tile_pool(name="x", bufs=9))
        actp = ctx.enter_context(tc.tile_pool(name="act", bufs=1))
        maskp = ctx.enter_context(tc.tile_pool(name="mask", bufs=2))
        upool = ctx.enter_context(tc.tile_pool(name="u", bufs=2))
        tpool = ctx.enter_context(tc.tile_pool(name="t", bufs=2))
        t2p = ctx.enter_context(tc.tile_pool(name="t2", bufs=2))
        recp = ctx.enter_context(tc.tile_pool(name="rec", bufs=3))
        outp = ctx.enter_context(tc.tile_pool(name="out", bufs=2))
        psB = ctx.enter_context(tc.tile_pool(name="psB", bufs=3, space="PSUM"))
        psS = ctx.enter_context(tc.tile_pool(name="psS", bufs=2, space="PSUM"))

        # ---- constants ----
        ones1 = const.tile([1, P], fp32)
        nc.any.memset(ones1[:], 1.0)
        nb1 = const.tile([1, 1], fp32)
        nc.sync.dma_start(nb1[:], nbias[:])
        # broadcast nbias to all 128 partitions via K=1 matmul outer product
        ps_nb = psS.tile([P, 512], fp32, tag="ps", name="psnb")
        nc.tensor.matmul(ps_nb[:, 0:1], ones1[:], nb1[:], start=True, stop=True)
        nb128 = const.tile([P, 1], fp32)
        nc.scalar.copy(nb128[:], ps_nb[:, 0:1])

        # ---- resident weights ----
        wq_sb = wpool.tile([P, DC, hs], bf16)
        wk_sb = wpool.tile([P, DC, hs], bf16)
        wv_sb = wpool.tile([P, DC, hs], bf16)
        wo_sb = wpool.tile([P, HC, d], bf16)
        for c in range(DC):
            nc.sync.dma_start(wq_sb[:, c], wqT_t[c])

        # ---- resident activations ----
        rqT_sb = actp.tile([P, HC, q], bf16)    # (hs, q)
        rkT_sb = actp.tile([P, HC, s], bf16)    # (hs, s)
        rv_sb = actp.tile([P, SC, hpg * (adim + 1)], bf16)  # (s, hs + ones)
        oT_sb = actp.tile([P, HC, q], bf16)     # (hs, q)
        nc.gpsimd.memset(rv_sb[:], 1.0)         # ones cols survive at 64::65

        # ---- stage A inputs (order on the sync queue matters: mask for the
        # first q tile is hoisted before the K-side loads so it cannot sit
        # behind xk transfers that wait on xq buffer reuse) ----
        xq = []
        for c in range(DC):
            xt = xpool.tile([P, q], bf16, tag="x", name=f"xq{c}")
            nc.sync.dma_start(xt[:], qT_t[c])
            xq.append(xt)
        mblk0 = maskp.tile([P, SC, qt], bf16, tag="m", name="mb0")
        nc.sync.dma_start(mblk0[:], maskT_t[:, :, 0:qt])
        for c in range(DC):
            nc.sync.dma_start(wk_sb[:, c], wkT_t[c])
            nc.sync.dma_start(wv_sb[:, c], wvT_t[c])
        xk = []
        for c in range(DC):
            xt = xpool.tile([P, s], bf16, tag="x", name=f"xk{c}")
            nc.sync.dma_start(xt[:], kT_t[c])
            xk.append(xt)
        for c in range(HC):
            nc.sync.dma_start(wo_sb[:, c], woT_t[c])

        def proj_block(m, w_sb, x_tiles, out_sb, on_act, sc_=1.0):
            """out_sb[:, m, :] = (W_m @ X) via weight-stationary matmuls.
            Matmul outputs are capped at one PSUM bank (512 fp32)."""
            for t2i in range(q // 1024):
                ps = psB.tile([P, 1024], fp32, tag="pb", name="pproj")
                for c in range(DC):
                    for j in (0, 1):
                        lo = t2i * 1024 + j * 512
                        nc.tensor.matmul(
                            ps[:, j * 512:(j + 1) * 512],
                            w_sb[:, c, m * P:(m + 1) * P],
                            x_tiles[c][:, lo:lo + 512],
                            start=(c == 0), stop=(c == DC - 1))
                sl = out_sb[:, m, t2i * 1024:(t2i + 1) * 1024]
                if on_act:
                    nc.scalar.activation(sl, ps[:], AF.Copy, scale=float(sc_))
                else:
                    nc.vector.tensor_copy(sl, ps[:])

        def rv_block():
            for sc in range(SC):
                ps = psS.tile([P, 512], fp32, tag="ps", name="prv")
                for c in range(DC):
                    nc.tensor.matmul(
                        ps[:, :hs], xk[c][:, sc * P:(sc + 1) * P],
                        wv_sb[:, c], start=(c == 0), stop=(c == DC - 1))
                nc.vector.tensor_copy(
                    rv_sb[:, sc].rearrange("p (h c) -> p h c", c=adim + 1)[:, :, 0:adim],
                    ps[:, :hs].rearrange("p (h c) -> p h c", c=adim))

        # ---- middle-phase blocks ----
        def scores_block(h, qlo, u):
            hp, hc = (h % 2) * adim, h // 2
            for k in range(8):          # sc pairs
                ps = psB.tile([P, 1024], fp32, tag="pb", name="pscore")
                for j in (0, 1):
                    sc = 2 * k + j
                    nc.tensor.matmul(
                        ps[:, j * 512:(j + 1) * 512],
                        rkT_sb[hp:hp + adim, hc, sc * P:(sc + 1) * P],
                        rqT_sb[hp:hp + adim, hc, qlo:qlo + qt],
                        start=True, stop=True)
                nc.scalar.activation(
                    u[:, 2 * k:2 * k + 2].rearrange("p a b -> p (a b)"),
                    ps[:], AF.Relu, bias=nb128[:])

        def ew_block(h, u, t, mblk):
            for j in range(4):          # 4-sc chunks
                u4 = u[:, 4 * j:4 * j + 4].rearrange("p a b -> p (a b)")
                m4 = mblk[:, 4 * j:4 * j + 4].rearrange("p a b -> p (a b)")
                t4 = t[:, 4 * j:4 * j + 4].rearrange("p a b -> p (a b)")
                t2 = t2p.tile([P, 2048], bf16, tag="t2", name="t2t")
                if USE_POW:
                    nc.vector.tensor_scalar(t2[:], u4, 2.0, None, ALU.pow)
                else:
                    nc.vector.tensor_mul(t2[:], u4, u4)
                eng = nc.gpsimd if j in POOL_CHUNKS else nc.vector
                eng.tensor_mul(t4, t2[:], m4)

        def av_mm(h, qlo, t):
            """AV matmuls + reciprocal; returns state for av_fin."""
            po = psS.tile([P, 512], fp32, tag="ps", name="pav")
            for sc in range(SC):
                nc.tensor.matmul(
                    po[0:adim + 1, :],
                    rv_sb[:, sc, h * (adim + 1):(h + 1) * (adim + 1)],
                    t[:, sc], start=(sc == 0), stop=(sc == SC - 1))
            rec = recp.tile([1, 512], fp32, tag="rec", name="rect")
            nc.vector.reciprocal(rec[:], po[adim:adim + 1, :])
            return (h, qlo, po, rec)

        def av_fin(h, qlo, po, rec):
            """Broadcast rec across partitions via PE, normalize + evict oT."""
            hp, hc = (h % 2) * adim, h // 2
            rb = psB.tile([P, 512], fp32, tag="pb", name="prb")
            nc.tensor.matmul(rb[0:adim, :], ones1[0:1, 0:adim], rec[:],
                             start=True, stop=True)
            rb_sb = recp.tile([adim, 512], fp32, tag="rb", name="rbt")
            nc.vector.tensor_copy(rb_sb[:], rb[0:adim, :])
            nc.vector.tensor_mul(oT_sb[hp:hp + adim, hc, qlo:qlo + qt],
                                 po[0:adim, :], rb_sb[:])

        def outproj(iq):
            qlo = iq * qt
            for qc in range(qt // P):
                ps = psB.tile([P, 1024], fp32, tag="pb", name="pout")
                for c in range(HC):
                    for j in (0, 1):
                        nc.tensor.matmul(
                            ps[:, j * 512:(j + 1) * 512],
                            oT_sb[:, c, qlo + qc * P:qlo + (qc + 1) * P],
                            wo_sb[:, c, j * 512:(j + 1) * 512],
                            start=(c == 0), stop=(c == HC - 1))
                ob = outp.tile([P, 1024], bf16, tag="ob", name="obt")
                if qc % 2 == 0:
                    nc.scalar.copy(ob[:], ps[:])
                else:
                    nc.vector.tensor_copy(ob[:], ps[:])
                nc.sync.dma_start(out_t[iq * (qt // P) + qc], ob[:])

        # ---- stage A. Both Q projections run first: Qm1 frees xq buffers
        # chunk-by-chunk, which the xk DMAs (xpool reuse) wait on, and Km0
        # consumes xk right behind them. Km1 + rv are interleaved into the
        # first q tile. ----
        proj_block(0, wq_sb, xq, rqT_sb, on_act=True, sc_=scale)
        proj_block(1, wq_sb, xq, rqT_sb, on_act=True, sc_=scale)
        proj_block(0, wk_sb, xk, rkT_sb, on_act=False)

        # ---- middle (AV pipelined two heads deep: mm at h-1, finish at h-2)
        prev = None           # (h, qlo, t) awaiting av_mm
        fin = None            # av_mm state awaiting av_fin
        prev_iq = None        # q tile awaiting out projection
        for iq in range(NQ):
            qlo = iq * qt
            if iq == 0:
                mblk = mblk0
            else:
                mblk = maskp.tile([P, SC, qt], bf16, tag="m", name=f"mb{iq}")
                nc.sync.dma_start(mblk[:], maskT_t[:, :, qlo:qlo + qt])
            for h in range(hpg):
                u = upool.tile([P, SC, qt], bf16, tag="u", name="ut")
                t = tpool.tile([P, SC, qt], bf16, tag="t", name="tt")
                scores_block(h, qlo, u)
                if iq == 0 and h == 1:
                    rv_block()
                nfin = av_mm(*prev) if prev is not None else None
                if fin is not None:
                    av_fin(*fin)
                fin = nfin
                if prev_iq is not None and h == 2:
                    outproj(prev_iq)
                    prev_iq = None
                ew_block(h, u, t, mblk)
                prev = (h, qlo, t)
                if iq == 0 and h == 0:
                    proj_block(1, wk_sb, xk, rkT_sb, on_act=False)
            prev_iq = iq
        av_fin(*fin)
        fin = av_mm(*prev)
        av_fin(*fin)
        outproj(prev_iq)

    nc.compile()
    return nc


def _shard_inputs(iQ, iK, mask, Wq, Wkv, Wo, nbias):
    in_maps = []
    maskT_by_b = [np.ascontiguousarray((~mask[b]).T).astype(BF16)
                  for b in range(B)]
    qT_by_b = [np.ascontiguousarray(iQ[b].T).astype(BF16) for b in range(B)]
    kT_by_b = [np.ascontiguousarray(iK[b].T).astype(BF16) for b in range(B)]
    nb = np.asarray(nbias, np.float32).reshape(1, 1)
    for ci in range(N_CORES):
        b, g = ci // GROUPS, ci % GROUPS
        hsl = slice(g * HS, (g + 1) * HS)
        in_maps.append({
            "qT": qT_by_b[b],
            "kT": kT_by_b[b],
            "wqT": np.ascontiguousarray(Wq[hsl].T).astype(BF16),
            "wkT": np.ascontiguousarray(Wkv[hsl].T).astype(BF16),
            "wvT": np.ascontiguousarray(Wkv[HSIZE + g * HS:HSIZE + (g + 1) * HS].T).astype(BF16),
            "woT": np.ascontiguousarray(Wo[:, hsl].T).astype(BF16),
            "maskT": maskT_by_b[b],
            "nbias": nb,
        })
    return in_maps


def kernel(iQ, iK, mask, Wq, Wkv, Wo, nbias):
    global _COMPILED
    from concourse.bass_utils import run_bass_kernel_spmd

    if _COMPILED is None:
        _COMPILED = _build()
    in_maps = _shard_inputs(np.asarray(iQ, np.float32), np.asarray(iK, np.float32),
                            np.asarray(mask), np.asarray(Wq, np.float32),
                            np.asarray(Wkv, np.float32), np.asarray(Wo, np.float32),
                            np.asarray(nbias, np.float32))
    res = run_bass_kernel_spmd(_COMPILED, in_maps, list(range(N_CORES))).results
    out = np.zeros((B, Q, D), np.float32)
    for ci in range(N_CORES):
        out[ci // GROUPS] += np.asarray(res[ci]["out"], np.float32)
    return out
